# revision 13
# baseline (speedup 1.0000x reference)
"""ArcFace layer distributed Bass kernel for 8 TRN2 NeuronCores.

Math (reference):
    emb_n = embedding / ||embedding||_row          [B, D]
    w_n   = kernel / ||kernel||_col                [D, C]
    cos   = emb_n @ w_n                            [B, C]
    out   = S*cos  everywhere except out[b, labels[b]] which gets the
            arcface margin value computed from cos[b, labels[b]].

Strategy (classification-parallel, per sharding hint):
  - shard kernel columns (classes) 8 ways (pad C=10572 -> 8*1328, plus one
    dummy column per shard used as a scatter target for out-of-range labels)
  - replicate embeddings (passed pre-transposed [D, B] so the contraction
    dim lands on SBUF partitions without an on-device transpose)
  - each core: column norms + row norms on device, normalized fp32 matmul,
    then an indirect-DMA gather/fix/scatter of the <=2048 label positions.

B=2048, D=512, C=10572, S=64, M=0.5.
"""

import math
import os

import numpy as np

os.environ.setdefault("MYCRO_LOCAL_CACHE", "1")

import concourse.bass as bass
import concourse.bacc as bacc
import concourse.mybir as mybir
import concourse.tile as tile
from concourse.bass_utils import run_bass_kernel_spmd

# ---------------- problem constants (hardcoded; kernel.py is standalone) ----
S = 64.0
MARGIN = 0.5
B = 2048          # batch
D = 512           # feature dim
C = 10572         # classes
NCORES = 8
SHARD = 1328      # real class columns per core (8*1328 = 10624 >= 10572)
W = SHARD + 1     # + dummy column for out-of-range label scatters
KT = D // 128     # 4 k-subtiles
MT = B // 128     # 16 m-tiles

COS_M = math.cos(MARGIN)
SIN_M = math.sin(MARGIN)
MM = SIN_M * MARGIN
THRESHOLD = math.cos(math.pi - MARGIN)

F32 = mybir.dt.float32
I32 = mybir.dt.int32

# N-chunks of the W axis (PSUM bank = 512 fp32)
NCHUNKS = []
_c0 = 0
while _c0 < W:
    _cn = min(512, W - _c0)
    NCHUNKS.append((_c0, _cn))
    _c0 += _cn


def build_nc() -> bass.Bass:
    nc = bacc.Bacc()
    embT_h = nc.declare_dram_parameter("embT", [D, B], F32, isOutput=False)
    w_h = nc.declare_dram_parameter("w", [D, W], F32, isOutput=False)
    offs_h = nc.declare_dram_parameter("offs", [B], I32, isOutput=False)
    out_h = nc.declare_dram_parameter("out", [B * W], F32, isOutput=True)

    out2d = out_h[:].rearrange("(b w) -> b w", w=W)
    out_flat = out_h[:, None]  # [B*W, 1] view for indirect gather/scatter


    with tile.TileContext(nc) as tc:
        with (
            tc.tile_pool(name="persist", bufs=1) as persist,
            tc.tile_pool(name="scratch", bufs=2) as scratch,
            tc.tile_pool(name="outp", bufs=3) as outp,
            tc.tile_pool(name="micro", bufs=2) as micro,
            tc.tile_pool(name="psum", bufs=2, space="PSUM") as psum,
        ):
            # ---------------- input DMAs ----------------
            et = persist.tile([128, KT, B], F32, tag="et")
            for kt in range(KT):
                nc.sync.dma_start(
                    et[:, kt, :], embT_h[kt * 128:(kt + 1) * 128, :]
                )
            wsb = persist.tile([128, KT, W], F32, tag="wsb")
            for kt in range(KT):
                nc.sync.dma_start(
                    wsb[:, kt, :], w_h[kt * 128:(kt + 1) * 128, :]
                )
            offs_sb = persist.tile([128, MT], I32, tag="offs")
            nc.sync.dma_start(offs_sb[:], offs_h.rearrange("(p j) -> p j", p=128))

            ones_col = persist.tile([128, 1], F32, tag="ones")
            nc.vector.memset(ones_col[:], 1.0)

            # ---------------- norms: sum of squares over the feature dim ---
            # squares accumulated in SBUF, then partition-reduced via a
            # ones-vector matmul (one matmul per 512-chunk, start=stop=True).
            ssq_e = persist.tile([128, B], F32, tag="ssq_e")
            nc.scalar.square(ssq_e[:], et[:, 0, :])
            for kt in range(1, KT):
                sq = scratch.tile([128, B], F32, tag="sq")
                nc.scalar.square(sq[:], et[:, kt, :])
                nc.vector.tensor_add(out=ssq_e[:], in0=ssq_e[:], in1=sq[:])

            ssq_w = persist.tile([128, W], F32, tag="ssq_w")
            nc.scalar.square(ssq_w[:], wsb[:, 0, :])
            for kt in range(1, KT):
                sqw = scratch.tile([128, W], F32, tag="sq")
                nc.scalar.square(sqw[:], wsb[:, kt, :])
                nc.vector.tensor_add(out=ssq_w[:], in0=ssq_w[:], in1=sqw[:])

            # rows: es_row[0, b] = S / ||emb_b||; ws_row[0, c] = 1 / ||w_c||
            es_row = persist.tile([1, B], F32, tag="es_row")
            for c0 in range(0, B, 512):
                nps = psum.tile([1, 512], F32, tag="nps")
                nc.tensor.matmul(
                    out=nps[:, :], lhsT=ones_col[:, :], rhs=ssq_e[:, c0:c0 + 512],
                    start=True, stop=True,
                )
                trow = micro.tile([1, 512], F32, tag="trow_e%d" % c0)
                nc.vector.reciprocal(trow[:, :], nps[:, :])
                nc.scalar.activation(
                    es_row[:, c0:c0 + 512], trow[:, :],
                    mybir.ActivationFunctionType.Sqrt, scale=S * S,
                )
            ws_row = persist.tile([1, W], F32, tag="ws_row")
            for (c0, cn) in NCHUNKS:
                nps = psum.tile([1, 512], F32, tag="nps")
                nc.tensor.matmul(
                    out=nps[:, :cn], lhsT=ones_col[:, :], rhs=ssq_w[:, c0:c0 + cn],
                    start=True, stop=True,
                )
                trow = micro.tile([1, 512], F32, tag="trow_w%d" % c0)
                nc.vector.reciprocal(trow[:, :cn], nps[:, :cn])
                nc.scalar.activation(
                    ws_row[:, c0:c0 + cn], trow[:, :cn],
                    mybir.ActivationFunctionType.Sqrt, scale=1.0,
                )

            # broadcast the row vectors across partitions via a K=1 ones
            # outer-product matmul (stride-0 DMA broadcasts fail NEFF load),
            # then normalize in place
            ones_row = persist.tile([1, 128], F32, tag="ones_row")
            nc.vector.memset(ones_row[:], 1.0)
            es_bc = persist.tile([128, B], F32, tag="es_bc")
            for c0 in range(0, B, 512):
                bps = psum.tile([128, 512], F32, tag="ps0", name="bps_e%d" % c0)
                nc.tensor.matmul(
                    out=bps[:, :], lhsT=ones_row[:, :], rhs=es_row[:, c0:c0 + 512],
                    start=True, stop=True,
                )
                nc.vector.tensor_copy(out=es_bc[:, c0:c0 + 512], in_=bps[:, :])
            ws_bc = persist.tile([128, W], F32, tag="ws_bc")
            for (c0, cn) in NCHUNKS:
                bps = psum.tile([128, 512], F32, tag="ps1", name="bps_w%d" % c0)
                nc.tensor.matmul(
                    out=bps[:, :cn], lhsT=ones_row[:, :], rhs=ws_row[:, c0:c0 + cn],
                    start=True, stop=True,
                )
                nc.vector.tensor_copy(out=ws_bc[:, c0:c0 + cn], in_=bps[:, :cn])
            for kt in range(KT):
                nc.vector.tensor_mul(
                    out=et[:, kt, :], in0=et[:, kt, :], in1=es_bc[:]
                )
            for kt in range(KT):
                nc.vector.tensor_mul(
                    out=wsb[:, kt, :], in0=wsb[:, kt, :], in1=ws_bc[:]
                )

            # ---------------- main matmul: out[m*128:(m+1)*128, :] ----------
            # Per m-tile: matmul -> epilogue copy -> out DMA -> label fixup.
            # HW indirect DMA honors ONE offset per partition ([128,1]), so
            # the gather/fix/scatter runs per m-tile (batch row b = m*128+p
            # maps to partition p), pipelined with later tiles' matmuls.
            for m in range(MT):
                pss = []
                for j, (c0, cn) in enumerate(NCHUNKS):
                    pss.append(
                        psum.tile([128, 512], F32, tag="ps%d" % j, name="ps%d_%d" % (j, m))
                    )
                for kt in range(KT):
                    lhsT = et[:, kt, m * 128:(m + 1) * 128]
                    for j, (c0, cn) in enumerate(NCHUNKS):
                        nc.tensor.matmul(
                            out=pss[j][:, :cn], lhsT=lhsT,
                            rhs=wsb[:, kt, c0:c0 + cn],
                            start=(kt == 0), stop=(kt == KT - 1),
                        )
                ot = outp.tile([128, W], F32, tag="ot", name="ot%d" % m)
                for j, (c0, cn) in enumerate(NCHUNKS):
                    nc.vector.tensor_copy(out=ot[:, c0:c0 + cn], in_=pss[j][:, :cn])
                nc.sync.dma_start(out2d[m * 128:(m + 1) * 128, :], ot[:])

                # ---- label fixup for this m-tile (g = S*cos at label) ----
                g = micro.tile([128, 1], F32, tag="g", name="g%d" % m)
                nc.gpsimd.indirect_dma_start(
                    out=g[:],
                    out_offset=None,
                    in_=out_flat,
                    in_offset=bass.IndirectOffsetOnAxis(
                        ap=offs_sb[:, m:m + 1], axis=0
                    ),
                )
                cos = micro.tile([128, 1], F32, tag="cos", name="cos%d" % m)
                nc.vector.tensor_scalar_mul(cos[:], g[:], 1.0 / S)
                om = micro.tile([128, 1], F32, tag="om", name="om%d" % m)
                nc.vector.tensor_tensor(
                    out=om[:], in0=cos[:], in1=cos[:], op=mybir.AluOpType.mult
                )
                nc.vector.tensor_scalar(
                    out=om[:], in0=om[:], scalar1=-1.0, scalar2=1.0,
                    op0=mybir.AluOpType.mult, op1=mybir.AluOpType.add,
                )
                nc.vector.tensor_scalar_max(om[:], om[:], 0.0)
                sin = micro.tile([128, 1], F32, tag="sin", name="sin%d" % m)
                nc.scalar.sqrt(sin[:], om[:])
                # cos_mt = g*cos_m - sin*(S*sin_m); keep = g - S*mm
                t2 = micro.tile([128, 1], F32, tag="t2", name="t2%d" % m)
                nc.vector.tensor_scalar_mul(t2[:], sin[:], S * SIN_M)
                cosmt = micro.tile([128, 1], F32, tag="cosmt", name="cosmt%d" % m)
                nc.vector.tensor_scalar_mul(cosmt[:], g[:], COS_M)
                nc.vector.tensor_sub(out=cosmt[:], in0=cosmt[:], in1=t2[:])
                keep = micro.tile([128, 1], F32, tag="keep", name="keep%d" % m)
                nc.vector.tensor_scalar_add(keep[:], g[:], -S * MM)
                mask = micro.tile([128, 1], mybir.dt.uint8, tag="mask", name="mask%d" % m)
                nc.vector.tensor_scalar(
                    out=mask[:], in0=g[:], scalar1=S * THRESHOLD, scalar2=None,
                    op0=mybir.AluOpType.is_gt,
                )
                val = micro.tile([128, 1], F32, tag="val", name="val%d" % m)
                nc.vector.select(val[:], mask[:], cosmt[:], keep[:])
                nc.gpsimd.indirect_dma_start(
                    out=out_flat,
                    out_offset=bass.IndirectOffsetOnAxis(
                        ap=offs_sb[:, m:m + 1], axis=0
                    ),
                    in_=val[:],
                    in_offset=None,
                )

    nc.finalize()
    return nc


_NC_CACHE: bass.Bass | None = None


def get_nc() -> bass.Bass:
    global _NC_CACHE
    if _NC_CACHE is None:
        _NC_CACHE = build_nc()
    return _NC_CACHE


def make_in_maps(embedding: np.ndarray, kernel: np.ndarray, labels: np.ndarray):
    embedding = np.asarray(embedding, dtype=np.float32)
    kernel = np.asarray(kernel, dtype=np.float32)
    labels = np.asarray(labels, dtype=np.int32)

    embT = np.ascontiguousarray(embedding.T)  # [D, B]
    # pad classes to 8*SHARD with ones-columns (nonzero so norms are finite)
    kern_pad = np.ones((D, NCORES * SHARD), dtype=np.float32)
    kern_pad[:, :C] = kernel

    in_maps = []
    for i in range(NCORES):
        wi = np.ones((D, W), dtype=np.float32)
        wi[:, :SHARD] = kern_pad[:, i * SHARD:(i + 1) * SHARD]
        # local label offsets: flat element offsets into the [B, W] output,
        # packed so SBUF tile [128, MT] holds offs for batch b = m*128 + p
        # at [p, m] (one offset per partition per indirect DMA).
        loc = labels - i * SHARD
        loc = np.where((loc >= 0) & (loc < SHARD), loc, SHARD).astype(np.int64)
        offs = (np.arange(B, dtype=np.int64) * W + loc).astype(np.int32)
        packed = np.ascontiguousarray(offs.reshape(MT, 128).T).ravel()
        in_maps.append(
            {"embT": embT, "w": np.ascontiguousarray(wi), "offs": packed}
        )
    return in_maps


def assemble(results) -> np.ndarray:
    parts = []
    for i in range(NCORES):
        o = np.asarray(results[i]["out"]).reshape(B, W)
        parts.append(o[:, :SHARD])
    return np.concatenate(parts, axis=1)[:, :C].astype(np.float32)


def kernel(embedding: np.ndarray, kernel: np.ndarray, labels: np.ndarray) -> np.ndarray:
    nc = get_nc()
    in_maps = make_in_maps(embedding, kernel, labels)
    res = run_bass_kernel_spmd(nc, in_maps, core_ids=list(range(NCORES)))
    return assemble(res.results)


if __name__ == "__main__":
    rng = np.random.default_rng(0)
    emb = rng.standard_normal((B, D), dtype=np.float32)
    kern = (rng.standard_normal((D, C), dtype=np.float32) * 0.05).astype(np.float32)
    labs = rng.integers(0, C, size=(B,), dtype=np.int32)
    out = kernel(emb, kern, labs)
    print(out.shape, out.dtype)


# revision 14
# speedup vs baseline: 1.1634x; 1.1634x over previous
"""ArcFace layer distributed Bass kernel for 8 TRN2 NeuronCores.

Math (reference):
    emb_n = embedding / ||embedding||_row          [B, D]
    w_n   = kernel / ||kernel||_col                [D, C]
    cos   = emb_n @ w_n                            [B, C]
    out   = S*cos  everywhere except out[b, labels[b]] which gets the
            arcface margin value computed from cos[b, labels[b]].

Strategy (classification-parallel, per sharding hint):
  - shard kernel columns (classes) 8 ways (pad C=10572 -> 8*1328, plus one
    dummy column per shard used as a scatter target for out-of-range labels)
  - replicate embeddings (passed pre-transposed [D, B] so the contraction
    dim lands on SBUF partitions without an on-device transpose)
  - each core: column norms + row norms on device, normalized fp32 matmul,
    then per-m-tile indirect-DMA gather/fix/scatter of the label positions.
    The output lives in 16 per-m-tile DRAM tensors so each tile's fixup only
    depends on that tile's output DMA (keeps the pipeline running).

B=2048, D=512, C=10572, S=64, M=0.5.
"""

import math
import os

import numpy as np

os.environ.setdefault("MYCRO_LOCAL_CACHE", "1")

import concourse.bass as bass
import concourse.bacc as bacc
import concourse.mybir as mybir
import concourse.tile as tile
from concourse.bass_utils import run_bass_kernel_spmd

# ---------------- problem constants (hardcoded; kernel.py is standalone) ----
S = 64.0
MARGIN = 0.5
B = 2048          # batch
D = 512           # feature dim
C = 10572         # classes
NCORES = 8
SHARD = 1328      # real class columns per core (8*1328 = 10624 >= 10572)
W = SHARD + 1     # + dummy column for out-of-range label scatters
KT = D // 128     # 4 k-subtiles
MT = B // 128     # 16 m-tiles
GRP = 4           # fixup math batched over GRP m-tiles

COS_M = math.cos(MARGIN)
SIN_M = math.sin(MARGIN)
MM = SIN_M * MARGIN
THRESHOLD = math.cos(math.pi - MARGIN)

F32 = mybir.dt.float32
I32 = mybir.dt.int32

# N-chunks of the W axis (PSUM bank = 512 fp32)
NCHUNKS = []
_c0 = 0
while _c0 < W:
    _cn = min(512, W - _c0)
    NCHUNKS.append((_c0, _cn))
    _c0 += _cn


def _emit_fixup_math(nc, micro, g, grp):
    """ArcFace margin on a [128, GRP] tile of gathered values g = S*cos."""
    om = micro.tile([128, GRP], F32, tag="om", name="om%d" % grp)
    # om = 1 - (g/S)^2  via (g * -1/S^2) * g + 1
    nc.vector.scalar_tensor_tensor(
        out=om[:], in0=g[:], scalar=-1.0 / (S * S), in1=g[:],
        op0=mybir.AluOpType.mult, op1=mybir.AluOpType.mult,
    )
    nc.vector.tensor_scalar_add(om[:], om[:], 1.0)
    nc.vector.tensor_scalar_max(om[:], om[:], 0.0)
    sin = micro.tile([128, GRP], F32, tag="sin", name="sin%d" % grp)
    nc.scalar.sqrt(sin[:], om[:])                      # ACT
    # cos_mt = g*cos_m - sin*(S*sin_m)
    cosmt = micro.tile([128, GRP], F32, tag="cosmt", name="cosmt%d" % grp)
    nc.vector.tensor_scalar_mul(cosmt[:], g[:], COS_M)
    nc.vector.scalar_tensor_tensor(
        out=cosmt[:], in0=sin[:], scalar=-S * SIN_M, in1=cosmt[:],
        op0=mybir.AluOpType.mult, op1=mybir.AluOpType.add,
    )
    # keep = g - S*mm
    keep = micro.tile([128, GRP], F32, tag="keep", name="keep%d" % grp)
    nc.vector.tensor_scalar_add(keep[:], g[:], -S * MM)
    mask = micro.tile([128, GRP], mybir.dt.uint8, tag="mask", name="mask%d" % grp)
    nc.vector.tensor_scalar(
        out=mask[:], in0=g[:], scalar1=S * THRESHOLD, scalar2=None,
        op0=mybir.AluOpType.is_gt,
    )
    val = micro.tile([128, GRP], F32, tag="val", name="val%d" % grp)
    nc.vector.select(val[:], mask[:], cosmt[:], keep[:])
    return val


def build_nc() -> bass.Bass:
    nc = bacc.Bacc()
    embT_h = nc.declare_dram_parameter("embT", [D, B], F32, isOutput=False)
    w_h = nc.declare_dram_parameter("w", [D, W], F32, isOutput=False)
    offs_h = nc.declare_dram_parameter("offs", [B], I32, isOutput=False)
    # one output tensor per m-tile: fixup of tile m then only depends on
    # tile m's own output DMA instead of serializing the whole output
    outs = [
        nc.declare_dram_parameter("out%d" % m, [128 * W], F32, isOutput=True)
        for m in range(MT)
    ]

    with tile.TileContext(nc) as tc:
        with (
            tc.tile_pool(name="persist", bufs=1) as persist,
            tc.tile_pool(name="scratch", bufs=2) as scratch,
            tc.tile_pool(name="outp", bufs=3) as outp,
            tc.tile_pool(name="micro", bufs=2) as micro,
            tc.tile_pool(name="psum", bufs=2, space="PSUM") as psum,
        ):
            # ---------------- input DMAs ----------------
            et = persist.tile([128, KT, B], F32, tag="et")
            for kt in range(KT):
                nc.sync.dma_start(
                    et[:, kt, :], embT_h[kt * 128:(kt + 1) * 128, :]
                )
            wsb = persist.tile([128, KT, W], F32, tag="wsb")
            for kt in range(KT):
                nc.sync.dma_start(
                    wsb[:, kt, :], w_h[kt * 128:(kt + 1) * 128, :]
                )
            offs_sb = persist.tile([128, MT], I32, tag="offs")
            nc.sync.dma_start(offs_sb[:], offs_h.rearrange("(p j) -> p j", p=128))

            ones_col = persist.tile([128, 1], F32, tag="ones")
            nc.vector.memset(ones_col[:], 1.0)

            # ---------------- norms: sum of squares over the feature dim ---
            ssq_e = persist.tile([128, B], F32, tag="ssq_e")
            nc.scalar.square(ssq_e[:], et[:, 0, :])
            for kt in range(1, KT):
                sq = scratch.tile([128, B], F32, tag="sq")
                nc.scalar.square(sq[:], et[:, kt, :])
                nc.vector.tensor_add(out=ssq_e[:], in0=ssq_e[:], in1=sq[:])

            ssq_w = persist.tile([128, W], F32, tag="ssq_w")
            nc.scalar.square(ssq_w[:], wsb[:, 0, :])
            for kt in range(1, KT):
                sqw = scratch.tile([128, W], F32, tag="sq")
                nc.scalar.square(sqw[:], wsb[:, kt, :])
                nc.vector.tensor_add(out=ssq_w[:], in0=ssq_w[:], in1=sqw[:])

            # rows: es_row[0, b] = S / ||emb_b||; ws_row[0, c] = 1 / ||w_c||
            es_row = persist.tile([1, B], F32, tag="es_row")
            for c0 in range(0, B, 512):
                nps = psum.tile([1, 512], F32, tag="nps")
                nc.tensor.matmul(
                    out=nps[:, :], lhsT=ones_col[:, :], rhs=ssq_e[:, c0:c0 + 512],
                    start=True, stop=True,
                )
                trow = micro.tile([1, 512], F32, tag="trow_e%d" % c0)
                nc.vector.reciprocal(trow[:, :], nps[:, :])
                nc.scalar.activation(
                    es_row[:, c0:c0 + 512], trow[:, :],
                    mybir.ActivationFunctionType.Sqrt, scale=S * S,
                )
            ws_row = persist.tile([1, W], F32, tag="ws_row")
            for (c0, cn) in NCHUNKS:
                nps = psum.tile([1, 512], F32, tag="nps")
                nc.tensor.matmul(
                    out=nps[:, :cn], lhsT=ones_col[:, :], rhs=ssq_w[:, c0:c0 + cn],
                    start=True, stop=True,
                )
                trow = micro.tile([1, 512], F32, tag="trow_w%d" % c0)
                nc.vector.reciprocal(trow[:, :cn], nps[:, :cn])
                nc.scalar.activation(
                    ws_row[:, c0:c0 + cn], trow[:, :cn],
                    mybir.ActivationFunctionType.Sqrt, scale=1.0,
                )

            # broadcast the row vectors across partitions via a K=1 ones
            # outer-product matmul (stride-0 DMA broadcasts fail NEFF load),
            # then normalize in place
            ones_row = persist.tile([1, 128], F32, tag="ones_row")
            nc.vector.memset(ones_row[:], 1.0)
            es_bc = persist.tile([128, B], F32, tag="es_bc")
            for c0 in range(0, B, 512):
                bps = psum.tile([128, 512], F32, tag="ps0", name="bps_e%d" % c0)
                nc.tensor.matmul(
                    out=bps[:, :], lhsT=ones_row[:, :], rhs=es_row[:, c0:c0 + 512],
                    start=True, stop=True,
                )
                nc.vector.tensor_copy(out=es_bc[:, c0:c0 + 512], in_=bps[:, :])
            ws_bc = persist.tile([128, W], F32, tag="ws_bc")
            for (c0, cn) in NCHUNKS:
                bps = psum.tile([128, 512], F32, tag="ps1", name="bps_w%d" % c0)
                nc.tensor.matmul(
                    out=bps[:, :cn], lhsT=ones_row[:, :], rhs=ws_row[:, c0:c0 + cn],
                    start=True, stop=True,
                )
                nc.vector.tensor_copy(out=ws_bc[:, c0:c0 + cn], in_=bps[:, :cn])
            for kt in range(KT):
                nc.vector.tensor_mul(
                    out=et[:, kt, :], in0=et[:, kt, :], in1=es_bc[:]
                )
            for kt in range(KT):
                nc.vector.tensor_mul(
                    out=wsb[:, kt, :], in0=wsb[:, kt, :], in1=ws_bc[:]
                )

            # ---------------- main loop over m-tiles ------------------------
            gtiles = {}
            for m in range(MT):
                pss = []
                for j, (c0, cn) in enumerate(NCHUNKS):
                    pss.append(
                        psum.tile([128, 512], F32, tag="ps%d" % j, name="ps%d_%d" % (j, m))
                    )
                for kt in range(KT):
                    lhsT = et[:, kt, m * 128:(m + 1) * 128]
                    for j, (c0, cn) in enumerate(NCHUNKS):
                        nc.tensor.matmul(
                            out=pss[j][:, :cn], lhsT=lhsT,
                            rhs=wsb[:, kt, c0:c0 + cn],
                            start=(kt == 0), stop=(kt == KT - 1),
                        )
                ot = outp.tile([128, W], F32, tag="ot", name="ot%d" % m)
                for j, (c0, cn) in enumerate(NCHUNKS):
                    nc.vector.tensor_copy(out=ot[:, c0:c0 + cn], in_=pss[j][:, :cn])
                out2d = outs[m][:].rearrange("(p w) -> p w", w=W)
                nc.sync.dma_start(out2d[:, :], ot[:])

                # gather this tile's label logits (one offset per partition)
                grp, gi = divmod(m, GRP)
                if gi == 0:
                    gtiles[grp] = micro.tile(
                        [128, GRP], F32, tag="g", name="g%d" % grp
                    )
                nc.gpsimd.indirect_dma_start(
                    out=gtiles[grp][:, gi:gi + 1],
                    out_offset=None,
                    in_=outs[m][:, None],
                    in_offset=bass.IndirectOffsetOnAxis(
                        ap=offs_sb[:, m:m + 1], axis=0
                    ),
                )
                if gi == GRP - 1:
                    val = _emit_fixup_math(nc, micro, gtiles[grp], grp)
                    for k in range(GRP):
                        mm_ = grp * GRP + k
                        nc.gpsimd.indirect_dma_start(
                            out=outs[mm_][:, None],
                            out_offset=bass.IndirectOffsetOnAxis(
                                ap=offs_sb[:, mm_:mm_ + 1], axis=0
                            ),
                            in_=val[:, k:k + 1],
                            in_offset=None,
                        )

    nc.finalize()
    return nc


_NC_CACHE: bass.Bass | None = None


def get_nc() -> bass.Bass:
    global _NC_CACHE
    if _NC_CACHE is None:
        _NC_CACHE = build_nc()
    return _NC_CACHE


def make_in_maps(embedding: np.ndarray, kernel: np.ndarray, labels: np.ndarray):
    embedding = np.asarray(embedding, dtype=np.float32)
    kernel = np.asarray(kernel, dtype=np.float32)
    labels = np.asarray(labels, dtype=np.int32)

    embT = np.ascontiguousarray(embedding.T)  # [D, B]
    # pad classes to 8*SHARD with ones-columns (nonzero so norms are finite)
    kern_pad = np.ones((D, NCORES * SHARD), dtype=np.float32)
    kern_pad[:, :C] = kernel

    in_maps = []
    for i in range(NCORES):
        wi = np.ones((D, W), dtype=np.float32)
        wi[:, :SHARD] = kern_pad[:, i * SHARD:(i + 1) * SHARD]
        # per-m-tile local offsets: batch b = m*128 + p lives in out{m} at
        # flat position p*W + loc[b]; SBUF tile [128, MT] holds it at [p, m].
        loc = labels - i * SHARD
        loc = np.where((loc >= 0) & (loc < SHARD), loc, SHARD).astype(np.int64)
        local = (np.arange(B, dtype=np.int64) % 128 + 0) * W + loc  # p*W + loc
        # b = m*128 + p  ->  p = b % 128?  No: p = b - m*128 = b % 128 only
        # because m-tiles are 128 consecutive rows; b % 128 == p. Correct.
        packed = np.ascontiguousarray(
            local.reshape(MT, 128).T
        ).ravel().astype(np.int32)
        in_maps.append(
            {"embT": embT, "w": np.ascontiguousarray(wi), "offs": packed}
        )
    return in_maps


def assemble(results) -> np.ndarray:
    parts = []
    for i in range(NCORES):
        rows = [
            np.asarray(results[i]["out%d" % m]).reshape(128, W)[:, :SHARD]
            for m in range(MT)
        ]
        parts.append(np.concatenate(rows, axis=0))
    return np.concatenate(parts, axis=1)[:, :C].astype(np.float32)


def kernel(embedding: np.ndarray, kernel: np.ndarray, labels: np.ndarray) -> np.ndarray:
    nc = get_nc()
    in_maps = make_in_maps(embedding, kernel, labels)
    res = run_bass_kernel_spmd(nc, in_maps, core_ids=list(range(NCORES)))
    return assemble(res.results)


if __name__ == "__main__":
    rng = np.random.default_rng(0)
    emb = rng.standard_normal((B, D), dtype=np.float32)
    kern = (rng.standard_normal((D, C), dtype=np.float32) * 0.05).astype(np.float32)
    labs = rng.integers(0, C, size=(B,), dtype=np.int32)
    out = kernel(emb, kern, labs)
    print(out.shape, out.dtype)


# revision 17
# speedup vs baseline: 2.0884x; 1.7951x over previous
"""ArcFace layer distributed Bass kernel for 8 TRN2 NeuronCores.

Math (reference):
    emb_n = embedding / ||embedding||_row          [B, D]
    w_n   = kernel / ||kernel||_col                [D, C]
    cos   = emb_n @ w_n                            [B, C]
    out   = S*cos  everywhere except out[b, labels[b]] which gets the
            arcface margin value computed from cos[b, labels[b]].

Strategy (classification-parallel, per sharding hint):
  - shard kernel columns (classes) 8 ways (pad C=10572 -> 8*1328, plus one
    dummy column per shard used as a scatter target for out-of-range labels)
  - replicate embeddings (pre-transposed [D, B] so the contraction dim lands
    on SBUF partitions); matmul operands in bf16 (fp32 accumulate, fp32 out)
  - matmuls run on RAW operands; both normalization scales are folded into
    the PSUM->SBUF epilogue:  ot = (psum * rs_e[row]) * ws[col]  (one
    scalar_tensor_tensor op per 512-chunk)
  - label fixup: per-m-tile indirect-DMA gather -> arcface margin -> scatter,
    on 16 per-m-tile output tensors so the fixups pipeline with the matmuls.

B=2048, D=512, C=10572, S=64, M=0.5.
"""

import math
import os

import numpy as np

os.environ.setdefault("MYCRO_LOCAL_CACHE", "1")

import concourse.bass as bass
import concourse.bacc as bacc
import concourse.mybir as mybir
import concourse.tile as tile
from concourse.bass_utils import run_bass_kernel_spmd

# ---------------- problem constants (hardcoded; kernel.py is standalone) ----
S = 64.0
MARGIN = 0.5
B = 2048          # batch
D = 512           # feature dim
C = 10572         # classes
NCORES = 8
SHARD = 1328      # real class columns per core (8*1328 = 10624 >= 10572)
W = SHARD + 1     # + dummy column for out-of-range label scatters
KT = D // 128     # 4 k-subtiles
MT = B // 128     # 16 m-tiles
GRP = 4           # fixup math batched over GRP m-tiles

COS_M = math.cos(MARGIN)
SIN_M = math.sin(MARGIN)
MM = SIN_M * MARGIN
THRESHOLD = math.cos(math.pi - MARGIN)

F32 = mybir.dt.float32
BF16 = mybir.dt.bfloat16
I32 = mybir.dt.int32

# N-chunks of the W axis (PSUM bank = 512 fp32)
NCHUNKS = []
_c0 = 0
while _c0 < W:
    _cn = min(512, W - _c0)
    NCHUNKS.append((_c0, _cn))
    _c0 += _cn


def _emit_fixup_math(nc, micro, g, grp):
    """ArcFace margin on a [128, GRP] tile of gathered values g = S*cos."""
    om = micro.tile([128, GRP], F32, tag="om", name="om%d" % grp)
    # om = 1 - (g/S)^2  via (g * -1/S^2) * g + 1
    nc.vector.scalar_tensor_tensor(
        out=om[:], in0=g[:], scalar=-1.0 / (S * S), in1=g[:],
        op0=mybir.AluOpType.mult, op1=mybir.AluOpType.mult,
    )
    nc.vector.tensor_scalar_add(om[:], om[:], 1.0)
    nc.vector.tensor_scalar_max(om[:], om[:], 0.0)
    sin = micro.tile([128, GRP], F32, tag="sin", name="sin%d" % grp)
    nc.scalar.sqrt(sin[:], om[:])                      # ACT
    # cos_mt = g*cos_m - sin*(S*sin_m)
    cosmt = micro.tile([128, GRP], F32, tag="cosmt", name="cosmt%d" % grp)
    nc.vector.tensor_scalar_mul(cosmt[:], g[:], COS_M)
    nc.vector.scalar_tensor_tensor(
        out=cosmt[:], in0=sin[:], scalar=-S * SIN_M, in1=cosmt[:],
        op0=mybir.AluOpType.mult, op1=mybir.AluOpType.add,
    )
    # keep = g - S*mm
    keep = micro.tile([128, GRP], F32, tag="keep", name="keep%d" % grp)
    nc.vector.tensor_scalar_add(keep[:], g[:], -S * MM)
    mask = micro.tile([128, GRP], mybir.dt.uint8, tag="mask", name="mask%d" % grp)
    nc.vector.tensor_scalar(
        out=mask[:], in0=g[:], scalar1=S * THRESHOLD, scalar2=None,
        op0=mybir.AluOpType.is_gt,
    )
    val = micro.tile([128, GRP], F32, tag="val", name="val%d" % grp)
    nc.vector.select(val[:], mask[:], cosmt[:], keep[:])
    return val


def build_nc() -> bass.Bass:
    nc = bacc.Bacc()
    w_h = nc.declare_dram_parameter("w", [D, W], BF16, isOutput=False)
    embT_h = nc.declare_dram_parameter("embT", [D, B], BF16, isOutput=False)
    offs_h = nc.declare_dram_parameter("offs", [B], I32, isOutput=False)
    # one output tensor per m-tile: fixup of tile m then only depends on
    # tile m's own output DMA instead of serializing the whole output
    outs = [
        nc.declare_dram_parameter("out%d" % m, [128 * W], F32, isOutput=True)
        for m in range(MT)
    ]


    with tile.TileContext(nc) as tc:
        with (
            tc.tile_pool(name="persist", bufs=1) as persist,
            tc.tile_pool(name="scratch", bufs=2) as scratch,
            tc.tile_pool(name="outp", bufs=3) as outp,
            tc.tile_pool(name="micro", bufs=2) as micro,
            tc.tile_pool(name="psum", bufs=2, space="PSUM") as psum,
        ):
            # ---------------- input DMAs (w first: its norm chain gates the
            # first epilogue together with the row scales) ----------------
            wsb = persist.tile([128, KT, W], BF16, tag="wsb")
            for kt in range(KT):
                nc.sync.dma_start(
                    wsb[:, kt, :], w_h[kt * 128:(kt + 1) * 128, :]
                )
            et = persist.tile([128, KT, B], BF16, tag="et")
            for kt in range(KT):
                nc.sync.dma_start(
                    et[:, kt, :], embT_h[kt * 128:(kt + 1) * 128, :]
                )
            offs_sb = persist.tile([128, MT], I32, tag="offs")
            nc.sync.dma_start(offs_sb[:], offs_h.rearrange("(p j) -> p j", p=128))

            ones_col = persist.tile([128, 1], F32, tag="ones")
            nc.vector.memset(ones_col[:], 1.0)
            ones_row = persist.tile([1, 128], F32, tag="ones_row")
            nc.vector.memset(ones_row[:], 1.0)

            # ---------------- norms: sum of squares over the feature dim ---
            ssq_w = persist.tile([128, W], F32, tag="ssq_w")
            nc.scalar.square(ssq_w[:], wsb[:, 0, :])
            for kt in range(1, KT):
                sqw = scratch.tile([128, W], F32, tag="sq")
                nc.scalar.square(sqw[:], wsb[:, kt, :])
                nc.vector.tensor_add(out=ssq_w[:], in0=ssq_w[:], in1=sqw[:])

            ssq_e = persist.tile([128, B], F32, tag="ssq_e")
            nc.scalar.square(ssq_e[:], et[:, 0, :])
            for kt in range(1, KT):
                sq = scratch.tile([128, B], F32, tag="sq")
                nc.scalar.square(sq[:], et[:, kt, :])
                nc.vector.tensor_add(out=ssq_e[:], in0=ssq_e[:], in1=sq[:])

            # ws_row[0, c] = 1/||w_c||, broadcast to ws_bc [128, W]
            ws_row = persist.tile([1, W], F32, tag="ws_row")
            for (c0, cn) in NCHUNKS:
                nps = psum.tile([1, 512], F32, tag="nps")
                nc.tensor.matmul(
                    out=nps[:, :cn], lhsT=ones_col[:, :], rhs=ssq_w[:, c0:c0 + cn],
                    start=True, stop=True,
                )
                trow = micro.tile([1, 512], F32, tag="trow_w%d" % c0)
                nc.vector.reciprocal(trow[:, :cn], nps[:, :cn])
                nc.scalar.activation(
                    ws_row[:, c0:c0 + cn], trow[:, :cn],
                    mybir.ActivationFunctionType.Sqrt, scale=1.0,
                )
            ws_bc = persist.tile([128, W], F32, tag="ws_bc")
            for (c0, cn) in NCHUNKS:
                bps = psum.tile([128, 512], F32, tag="ps0", name="bps_w%d" % c0)
                nc.tensor.matmul(
                    out=bps[:, :cn], lhsT=ones_row[:, :], rhs=ws_row[:, c0:c0 + cn],
                    start=True, stop=True,
                )
                nc.vector.tensor_copy(out=ws_bc[:, c0:c0 + cn], in_=bps[:, :cn])

            # es_row[0, b] = S/||emb_b||  -> DRAM bounce -> rs_em [128, MT]
            # (rs_em[p, m] = es_row[m*128 + p]: per-partition scale for the
            # m-th output tile's epilogue)
            es_row = persist.tile([1, B], F32, tag="es_row")
            for c0 in range(0, B, 512):
                nps = psum.tile([1, 512], F32, tag="nps")
                nc.tensor.matmul(
                    out=nps[:, :], lhsT=ones_col[:, :], rhs=ssq_e[:, c0:c0 + 512],
                    start=True, stop=True,
                )
                trow = micro.tile([1, 512], F32, tag="trow_e%d" % c0)
                nc.vector.reciprocal(trow[:, :], nps[:, :])
                nc.scalar.activation(
                    es_row[:, c0:c0 + 512], trow[:, :],
                    mybir.ActivationFunctionType.Sqrt, scale=S * S,
                )
            # redistribute es_row [1, B] -> rs_em [128, MT] with 16 tiny
            # K=1,N=1 matmuls (out[:, m] = es_row[0, m*128:(m+1)*128].T)
            one_one = persist.tile([1, 1], F32, tag="one_one")
            nc.vector.memset(one_one[:], 1.0)
            rps = psum.tile([128, MT], F32, tag="nps", name="rps")
            for m in range(MT):
                nc.tensor.matmul(
                    out=rps[:, m:m + 1],
                    lhsT=es_row[:, m * 128:(m + 1) * 128],
                    rhs=one_one[:, :],
                    start=True, stop=True,
                )
            rs_em = persist.tile([128, MT], F32, tag="rs_em")
            nc.vector.tensor_copy(out=rs_em[:], in_=rps[:])

            # ---------------- main loop over m-tiles ------------------------
            gtiles = {}
            for m in range(MT):
                pss = []
                for j, (c0, cn) in enumerate(NCHUNKS):
                    pss.append(
                        psum.tile([128, 512], F32, tag="ps%d" % j, name="ps%d_%d" % (j, m))
                    )
                for kt in range(KT):
                    lhsT = et[:, kt, m * 128:(m + 1) * 128]
                    for j, (c0, cn) in enumerate(NCHUNKS):
                        nc.tensor.matmul(
                            out=pss[j][:, :cn], lhsT=lhsT,
                            rhs=wsb[:, kt, c0:c0 + cn],
                            start=(kt == 0), stop=(kt == KT - 1),
                        )
                ot = outp.tile([128, W], F32, tag="ot", name="ot%d" % m)
                for j, (c0, cn) in enumerate(NCHUNKS):
                    # ot = (psum * rs_e[row]) * ws[col]
                    nc.vector.scalar_tensor_tensor(
                        out=ot[:, c0:c0 + cn], in0=pss[j][:, :cn],
                        scalar=rs_em[:, m:m + 1], in1=ws_bc[:, c0:c0 + cn],
                        op0=mybir.AluOpType.mult, op1=mybir.AluOpType.mult,
                    )
                out2d = outs[m][:].rearrange("(p w) -> p w", w=W)
                nc.sync.dma_start(out2d[:, :], ot[:])

                # gather this tile's label logits (one offset per partition)
                grp, gi = divmod(m, GRP)
                if gi == 0:
                    gtiles[grp] = micro.tile(
                        [128, GRP], F32, tag="g", name="g%d" % grp
                    )
                nc.gpsimd.indirect_dma_start(
                    out=gtiles[grp][:, gi:gi + 1],
                    out_offset=None,
                    in_=outs[m][:, None],
                    in_offset=bass.IndirectOffsetOnAxis(
                        ap=offs_sb[:, m:m + 1], axis=0
                    ),
                )
                if gi == GRP - 1:
                    val = _emit_fixup_math(nc, micro, gtiles[grp], grp)
                    for k in range(GRP):
                        mm_ = grp * GRP + k
                        nc.gpsimd.indirect_dma_start(
                            out=outs[mm_][:, None],
                            out_offset=bass.IndirectOffsetOnAxis(
                                ap=offs_sb[:, mm_:mm_ + 1], axis=0
                            ),
                            in_=val[:, k:k + 1],
                            in_offset=None,
                        )

    nc.finalize()
    return nc


_NC_CACHE: bass.Bass | None = None


def get_nc() -> bass.Bass:
    global _NC_CACHE
    if _NC_CACHE is None:
        _NC_CACHE = build_nc()
    return _NC_CACHE


def make_in_maps(embedding: np.ndarray, kernel: np.ndarray, labels: np.ndarray):
    embedding = np.asarray(embedding, dtype=np.float32)
    kernel = np.asarray(kernel, dtype=np.float32)
    labels = np.asarray(labels, dtype=np.int32)

    import ml_dtypes

    embT = np.ascontiguousarray(embedding.T).astype(ml_dtypes.bfloat16)
    kern_pad = np.ones((D, NCORES * SHARD), dtype=np.float32)
    kern_pad[:, :C] = kernel

    in_maps = []
    for i in range(NCORES):
        wi = np.ones((D, W), dtype=np.float32)
        wi[:, :SHARD] = kern_pad[:, i * SHARD:(i + 1) * SHARD]
        # per-m-tile local offsets: batch b = m*128 + p lives in out{m} at
        # flat position p*W + loc[b]; SBUF tile [128, MT] holds it at [p, m].
        loc = labels - i * SHARD
        loc = np.where((loc >= 0) & (loc < SHARD), loc, SHARD).astype(np.int64)
        local = (np.arange(B, dtype=np.int64) % 128) * W + loc
        packed = np.ascontiguousarray(
            local.reshape(MT, 128).T
        ).ravel().astype(np.int32)
        in_maps.append(
            {
                "embT": embT,
                "w": np.ascontiguousarray(wi).astype(ml_dtypes.bfloat16),
                "offs": packed,
            }
        )
    return in_maps


def assemble(results) -> np.ndarray:
    parts = []
    for i in range(NCORES):
        rows = [
            np.asarray(results[i]["out%d" % m]).reshape(128, W)[:, :SHARD]
            for m in range(MT)
        ]
        parts.append(np.concatenate(rows, axis=0))
    return np.concatenate(parts, axis=1)[:, :C].astype(np.float32)


def kernel(embedding: np.ndarray, kernel: np.ndarray, labels: np.ndarray) -> np.ndarray:
    nc = get_nc()
    in_maps = make_in_maps(embedding, kernel, labels)
    res = run_bass_kernel_spmd(nc, in_maps, core_ids=list(range(NCORES)))
    return assemble(res.results)


if __name__ == "__main__":
    rng = np.random.default_rng(0)
    emb = rng.standard_normal((B, D), dtype=np.float32)
    kern = (rng.standard_normal((D, C), dtype=np.float32) * 0.05).astype(np.float32)
    labs = rng.integers(0, C, size=(B,), dtype=np.int32)
    out = kernel(emb, kern, labs)
    print(out.shape, out.dtype)


# revision 22
# speedup vs baseline: 2.1232x; 1.0167x over previous
"""ArcFace layer distributed Bass kernel for 8 TRN2 NeuronCores.

Math (reference):
    emb_n = embedding / ||embedding||_row          [B, D]
    w_n   = kernel / ||kernel||_col                [D, C]
    cos   = emb_n @ w_n                            [B, C]
    out   = S*cos  everywhere except out[b, labels[b]] which gets the
            arcface margin value computed from cos[b, labels[b]].

Strategy (classification-parallel, per sharding hint):
  - shard kernel columns (classes) 8 ways (pad C=10572 -> 8*1328, plus one
    dummy column per shard used as a scatter target for out-of-range labels)
  - replicate embeddings (pre-transposed [D, B] so the contraction dim lands
    on SBUF partitions); matmul operands in bf16 (fp32 accumulate, fp32 out)
  - matmuls run on RAW operands; both normalization scales are folded into
    the PSUM->SBUF epilogue:  ot = (psum * rs_e[row]) * ws[col]  (one
    scalar_tensor_tensor op per 512-chunk)
  - label fixup: per-m-tile indirect-DMA gather -> arcface margin -> scatter,
    on 16 per-m-tile output tensors so the fixups pipeline with the matmuls.

B=2048, D=512, C=10572, S=64, M=0.5.
"""

import math
import os

import numpy as np

os.environ.setdefault("MYCRO_LOCAL_CACHE", "1")

import concourse.bass as bass
import concourse.bacc as bacc
import concourse.mybir as mybir
import concourse.tile as tile
from concourse.bass_utils import run_bass_kernel_spmd

# ---------------- problem constants (hardcoded; kernel.py is standalone) ----
S = 64.0
MARGIN = 0.5
B = 2048          # batch
D = 512           # feature dim
C = 10572         # classes
NCORES = 8
SHARD = 1328      # real class columns per core (8*1328 = 10624 >= 10572)
W = SHARD + 1     # + dummy column for out-of-range label scatters
KT = D // 128     # 4 k-subtiles
MT = B // 128     # 16 m-tiles
GRP = 4           # fixup math batched over GRP m-tiles

COS_M = math.cos(MARGIN)
SIN_M = math.sin(MARGIN)
MM = SIN_M * MARGIN
THRESHOLD = math.cos(math.pi - MARGIN)

F32 = mybir.dt.float32
BF16 = mybir.dt.bfloat16
I32 = mybir.dt.int32

# N-chunks of the W axis (PSUM bank = 512 fp32)
NCHUNKS = []
_c0 = 0
while _c0 < W:
    _cn = min(512, W - _c0)
    NCHUNKS.append((_c0, _cn))
    _c0 += _cn


def _emit_fixup_math(nc, micro, g, grp):
    """ArcFace margin on a [128, GRP] tile of gathered values g = S*cos."""
    om = micro.tile([128, GRP], F32, tag="om", name="om%d" % grp)
    # om = 1 - (g/S)^2  via (g * -1/S^2) * g + 1
    nc.vector.scalar_tensor_tensor(
        out=om[:], in0=g[:], scalar=-1.0 / (S * S), in1=g[:],
        op0=mybir.AluOpType.mult, op1=mybir.AluOpType.mult,
    )
    nc.vector.tensor_scalar_add(om[:], om[:], 1.0)
    nc.vector.tensor_scalar_max(om[:], om[:], 0.0)
    sin = micro.tile([128, GRP], F32, tag="sin", name="sin%d" % grp)
    nc.scalar.sqrt(sin[:], om[:])                      # ACT
    # cos_mt = g*cos_m - sin*(S*sin_m)
    cosmt = micro.tile([128, GRP], F32, tag="cosmt", name="cosmt%d" % grp)
    nc.vector.tensor_scalar_mul(cosmt[:], g[:], COS_M)
    nc.vector.scalar_tensor_tensor(
        out=cosmt[:], in0=sin[:], scalar=-S * SIN_M, in1=cosmt[:],
        op0=mybir.AluOpType.mult, op1=mybir.AluOpType.add,
    )
    # keep = g - S*mm
    keep = micro.tile([128, GRP], F32, tag="keep", name="keep%d" % grp)
    nc.vector.tensor_scalar_add(keep[:], g[:], -S * MM)
    mask = micro.tile([128, GRP], mybir.dt.uint8, tag="mask", name="mask%d" % grp)
    nc.vector.tensor_scalar(
        out=mask[:], in0=g[:], scalar1=S * THRESHOLD, scalar2=None,
        op0=mybir.AluOpType.is_gt,
    )
    val = micro.tile([128, GRP], F32, tag="val", name="val%d" % grp)
    nc.vector.select(val[:], mask[:], cosmt[:], keep[:])
    return val


def build_nc() -> bass.Bass:
    nc = bacc.Bacc()
    w_h = nc.declare_dram_parameter("w", [D, W], BF16, isOutput=False)
    embT_h = nc.declare_dram_parameter("embT", [D, B], BF16, isOutput=False)
    offs_h = nc.declare_dram_parameter("offs", [B], I32, isOutput=False)
    # one output tensor per m-tile: fixup of tile m then only depends on
    # tile m's own output DMA instead of serializing the whole output
    outs = [
        nc.declare_dram_parameter("out%d" % m, [128 * W], F32, isOutput=True)
        for m in range(MT)
    ]


    with tile.TileContext(nc) as tc:
        with (
            tc.tile_pool(name="persist", bufs=1) as persist,
            tc.tile_pool(name="scratch", bufs=2) as scratch,
            tc.tile_pool(name="outp", bufs=3) as outp,
            tc.tile_pool(name="micro", bufs=2) as micro,
            tc.tile_pool(name="psum", bufs=2, space="PSUM") as psum,
        ):
            # ---------------- input DMAs (interleaved so both norm chains
            # and the first matmuls can start early) ----------------
            wsb = persist.tile([128, KT, W], BF16, tag="wsb")
            et = persist.tile([128, KT, B], BF16, tag="et")
            for kt in range(KT):
                nc.sync.dma_start(
                    wsb[:, kt, :], w_h[kt * 128:(kt + 1) * 128, :]
                )
                nc.sync.dma_start(
                    et[:, kt, :], embT_h[kt * 128:(kt + 1) * 128, :]
                )
            offs_sb = persist.tile([128, MT], I32, tag="offs")
            nc.sync.dma_start(offs_sb[:], offs_h.rearrange("(p j) -> p j", p=128))

            ones_col = persist.tile([128, 1], F32, tag="ones")
            nc.vector.memset(ones_col[:], 1.0)
            ones_row = persist.tile([1, 128], F32, tag="ones_row")
            nc.vector.memset(ones_row[:], 1.0)
            one_one = persist.tile([1, 1], F32, tag="one_one")
            nc.vector.memset(one_one[:], 1.0)

            # ---------------- norms: sum of squares over the feature dim ---
            ssq_e = persist.tile([128, B], F32, tag="ssq_e")
            nc.scalar.square(ssq_e[:], et[:, 0, :])
            for kt in range(1, KT):
                sq = scratch.tile([128, B], F32, tag="sq", name="sqe%d" % kt)
                nc.scalar.square(sq[:], et[:, kt, :])
                nc.vector.tensor_add(out=ssq_e[:], in0=ssq_e[:], in1=sq[:])

            ssq_w = persist.tile([128, W], F32, tag="ssq_w")
            nc.scalar.square(ssq_w[:], wsb[:, 0, :])
            for kt in range(1, KT):
                sqw = scratch.tile([128, W], F32, tag="sq", name="sqw%d" % kt)
                nc.scalar.square(sqw[:], wsb[:, kt, :])
                nc.vector.tensor_add(out=ssq_w[:], in0=ssq_w[:], in1=sqw[:])

            # -- row scales: partition-reduce ssq_e, redistribute to
            # [128, MT], THEN rsqrt on 128 lanes: rs_em = S/||emb_row||
            essq_row = persist.tile([1, B], F32, tag="essq_row")
            for c0 in range(0, B, 512):
                nps = psum.tile([1, 512], F32, tag="nps", name="npse%d" % c0)
                nc.tensor.matmul(
                    out=nps[:, :], lhsT=ones_col[:, :], rhs=ssq_e[:, c0:c0 + 512],
                    start=True, stop=True,
                )
                nc.vector.tensor_copy(out=essq_row[:, c0:c0 + 512], in_=nps[:, :])
            rps = psum.tile([128, MT], F32, tag="nps", name="rps")
            for m in range(MT):
                nc.tensor.matmul(
                    out=rps[:, m:m + 1],
                    lhsT=essq_row[:, m * 128:(m + 1) * 128],
                    rhs=one_one[:, :],
                    start=True, stop=True,
                )
            rs_tmp = persist.tile([128, MT], F32, tag="rs_tmp")
            nc.vector.reciprocal(rs_tmp[:], rps[:])
            rs_em = persist.tile([128, MT], F32, tag="rs_em")
            nc.scalar.activation(
                rs_em[:], rs_tmp[:],
                mybir.ActivationFunctionType.Sqrt, scale=S * S,
            )

            # -- col scales: partition-reduce ssq_w, broadcast, THEN rsqrt
            # on 128 partitions: ws_bc[:, c] = 1/||w_col||
            wssq_row = persist.tile([1, W], F32, tag="wssq_row")
            for (c0, cn) in NCHUNKS:
                nps = psum.tile([1, 512], F32, tag="nps", name="npsw%d" % c0)
                nc.tensor.matmul(
                    out=nps[:, :cn], lhsT=ones_col[:, :], rhs=ssq_w[:, c0:c0 + cn],
                    start=True, stop=True,
                )
                nc.vector.tensor_copy(out=wssq_row[:, c0:c0 + cn], in_=nps[:, :cn])
            ws_bc = persist.tile([128, W], F32, tag="ws_bc")
            for (c0, cn) in NCHUNKS:
                bps = psum.tile([128, 512], F32, tag="ps0", name="bps_w%d" % c0)
                nc.tensor.matmul(
                    out=bps[:, :cn], lhsT=ones_row[:, :], rhs=wssq_row[:, c0:c0 + cn],
                    start=True, stop=True,
                )
                wtmp = scratch.tile([128, 512], F32, tag="wtmp", name="wtmp%d" % c0)
                nc.vector.reciprocal(wtmp[:, :cn], bps[:, :cn])
                nc.scalar.activation(
                    ws_bc[:, c0:c0 + cn], wtmp[:, :cn],
                    mybir.ActivationFunctionType.Sqrt, scale=1.0,
                )
            # pre-scale w by its column norms (in place, bf16) so the
            # epilogue needs only the per-partition row scale
            for kt in range(KT):
                nc.vector.tensor_mul(
                    out=wsb[:, kt, :], in0=wsb[:, kt, :], in1=ws_bc[:]
                )

            # ---------------- main loop over m-tiles ------------------------
            gtiles = {}
            for m in range(MT):
                pss = []
                for j, (c0, cn) in enumerate(NCHUNKS):
                    pss.append(
                        psum.tile([128, 512], F32, tag="ps%d" % j, name="ps%d_%d" % (j, m))
                    )
                for kt in range(KT):
                    lhsT = et[:, kt, m * 128:(m + 1) * 128]
                    for j, (c0, cn) in enumerate(NCHUNKS):
                        nc.tensor.matmul(
                            out=pss[j][:, :cn], lhsT=lhsT,
                            rhs=wsb[:, kt, c0:c0 + cn],
                            start=(kt == 0), stop=(kt == KT - 1),
                        )
                ot = outp.tile([128, W], F32, tag="ot", name="ot%d" % m)
                for j, (c0, cn) in enumerate(NCHUNKS):
                    # ot = psum * rs_e[row]; last (short) chunk goes to the
                    # scalar engine to offload DVE
                    if j < 2:
                        nc.vector.tensor_scalar_mul(
                            ot[:, c0:c0 + cn], pss[j][:, :cn], rs_em[:, m:m + 1]
                        )
                    else:
                        nc.scalar.activation(
                            ot[:, c0:c0 + cn], pss[j][:, :cn],
                            mybir.ActivationFunctionType.Copy,
                            scale=rs_em[:, m:m + 1],
                        )
                out2d = outs[m][:].rearrange("(p w) -> p w", w=W)
                nc.sync.dma_start(out2d[:, :], ot[:])

                # gather this tile's label logits (one offset per partition)
                grp, gi = divmod(m, GRP)
                if gi == 0:
                    gtiles[grp] = micro.tile(
                        [128, GRP], F32, tag="g", name="g%d" % grp
                    )
                nc.gpsimd.indirect_dma_start(
                    out=gtiles[grp][:, gi:gi + 1],
                    out_offset=None,
                    in_=outs[m][:, None],
                    in_offset=bass.IndirectOffsetOnAxis(
                        ap=offs_sb[:, m:m + 1], axis=0
                    ),
                )
                if gi == GRP - 1:
                    val = _emit_fixup_math(nc, micro, gtiles[grp], grp)
                    for k in range(GRP):
                        mm_ = grp * GRP + k
                        nc.gpsimd.indirect_dma_start(
                            out=outs[mm_][:, None],
                            out_offset=bass.IndirectOffsetOnAxis(
                                ap=offs_sb[:, mm_:mm_ + 1], axis=0
                            ),
                            in_=val[:, k:k + 1],
                            in_offset=None,
                        )

    nc.finalize()
    return nc


_NC_CACHE: bass.Bass | None = None


def get_nc() -> bass.Bass:
    global _NC_CACHE
    if _NC_CACHE is None:
        _NC_CACHE = build_nc()
    return _NC_CACHE


def make_in_maps(embedding: np.ndarray, kernel: np.ndarray, labels: np.ndarray):
    embedding = np.asarray(embedding, dtype=np.float32)
    kernel = np.asarray(kernel, dtype=np.float32)
    labels = np.asarray(labels, dtype=np.int32)

    import ml_dtypes

    embT = np.ascontiguousarray(embedding.T).astype(ml_dtypes.bfloat16)
    kern_pad = np.ones((D, NCORES * SHARD), dtype=np.float32)
    kern_pad[:, :C] = kernel

    in_maps = []
    for i in range(NCORES):
        wi = np.ones((D, W), dtype=np.float32)
        wi[:, :SHARD] = kern_pad[:, i * SHARD:(i + 1) * SHARD]
        # per-m-tile local offsets: batch b = m*128 + p lives in out{m} at
        # flat position p*W + loc[b]; SBUF tile [128, MT] holds it at [p, m].
        loc = labels - i * SHARD
        loc = np.where((loc >= 0) & (loc < SHARD), loc, SHARD).astype(np.int64)
        local = (np.arange(B, dtype=np.int64) % 128) * W + loc
        packed = np.ascontiguousarray(
            local.reshape(MT, 128).T
        ).ravel().astype(np.int32)
        in_maps.append(
            {
                "embT": embT,
                "w": np.ascontiguousarray(wi).astype(ml_dtypes.bfloat16),
                "offs": packed,
            }
        )
    return in_maps


def assemble(results) -> np.ndarray:
    parts = []
    for i in range(NCORES):
        rows = [
            np.asarray(results[i]["out%d" % m]).reshape(128, W)[:, :SHARD]
            for m in range(MT)
        ]
        parts.append(np.concatenate(rows, axis=0))
    return np.concatenate(parts, axis=1)[:, :C].astype(np.float32)


def kernel(embedding: np.ndarray, kernel: np.ndarray, labels: np.ndarray) -> np.ndarray:
    nc = get_nc()
    in_maps = make_in_maps(embedding, kernel, labels)
    res = run_bass_kernel_spmd(nc, in_maps, core_ids=list(range(NCORES)))
    return assemble(res.results)


if __name__ == "__main__":
    rng = np.random.default_rng(0)
    emb = rng.standard_normal((B, D), dtype=np.float32)
    kern = (rng.standard_normal((D, C), dtype=np.float32) * 0.05).astype(np.float32)
    labs = rng.integers(0, C, size=(B,), dtype=np.int32)
    out = kernel(emb, kern, labs)
    print(out.shape, out.dtype)
